# revision 1
# baseline (speedup 1.0000x reference)
"""CRNN (conv3x3 -> ReLU -> freq-maxpool -> GRU scan -> FC) on 8 Trainium2
NeuronCores, data-parallel over batch (8 items per core).

Structure per core:
  - conv: banded-weight matmuls over the frequency contraction; time shifts
    via column offsets into a padded fp32r tile; two accumulating matmuls per
    f-pair give PSUM [128 = 2f x 64c, 512t]; running tensor_max over f-pairs
    + ReLU(+bias) writes feat[c, t] batch-interleaved into bigU[64:128].
  - xn = W_ihn @ feat + b_ihn precomputed (PE), packed into bigH[64:128].
  - GRU scan with u/v decomposition: h_{k+1} = u_k + v_k, u_k = z_k*h_k,
    v_k = (1-z_k)*n_k. The rz matmul takes [u; feat] (K=128) plus a separate
    v matmul (K=64), so the only late operand on the serial chain is v.
  - FC from bigH h-history, output DMA'd straight from PSUM.
  - The time-half-1 conv work, the second half of xn, and the FC tiles are
    emitted interleaved with the scan steps so they execute in the scan's
    idle engine slots.
"""

import contextlib
import numpy as np

import concourse.bass as bass
import concourse.mybir as mybir
import concourse.tile as tile
from concourse import bacc
from concourse.bass_utils import run_bass_kernel_spmd

F32 = mybir.dt.float32
F32R = mybir.dt.float32r
AF = mybir.ActivationFunctionType
OP = mybir.AluOpType

B, F, T = 64, 64, 1024
C = 64
H = 64
OUT = 2
NCORES = 8
NB = B // NCORES
NFP = F // 2


def build_crnn(nb=NB, t_steps=T, reps=1, phases=("conv", "xn", "scan", "fc"),
               interleave=True):
    nc = bacc.Bacc("TRN2", target_bir_lowering=False, debug=False)
    TB = t_steps * nb
    NTH = max(1, t_steps // 512)
    THW = min(512, t_steps)
    NJ = max(1, TB // 512)
    JW = min(512, TB)
    full = len(phases) == 4
    inter = interleave and full and t_steps == T

    x_d = nc.declare_dram_parameter("x", [nb, F, t_steps], F32, isOutput=False)
    convA_d = nc.declare_dram_parameter("convA", [128, NFP * 128], F32, isOutput=False)
    convB_d = nc.declare_dram_parameter("convB", [64, NFP * 128], F32, isOutput=False)
    cb_d = nc.declare_dram_parameter("conv_bias", [C, 1], F32, isOutput=False)
    wrz_d = nc.declare_dram_parameter("w_rz_lhsT", [128, 128], F32, isOutput=False)
    wn_d = nc.declare_dram_parameter("w_n_lhsT", [H, H], F32, isOutput=False)
    win_d = nc.declare_dram_parameter("w_in_lhsT", [C, H], F32, isOutput=False)
    brz_d = nc.declare_dram_parameter("b_rz", [128, 1], F32, isOutput=False)
    brzn_d = nc.declare_dram_parameter("b_rz_neg", [H, 1], F32, isOutput=False)
    bhn_d = nc.declare_dram_parameter("b_hn", [H, 1], F32, isOutput=False)
    bin_d = nc.declare_dram_parameter("b_in_row", [1, H], F32, isOutput=False)
    fcw_d = nc.declare_dram_parameter("fc_lhsT", [H, OUT], F32, isOutput=False)
    fcb_d = nc.declare_dram_parameter("fc_b_row", [1, OUT], F32, isOutput=False)
    out_d = nc.declare_dram_parameter("out", [nb, OUT, t_steps], F32, isOutput=True)

    with tile.TileContext(nc) as tc:
        with (
            tc.tile_pool(name="persist", bufs=1) as persist,
            tc.tile_pool(name="stage", bufs=2) as stage,
            tc.tile_pool(name="x2pool", bufs=1) as x2p,
            tc.tile_pool(name="work", bufs=2) as work,
            tc.tile_pool(name="scanw", bufs=3) as scanw,
            tc.tile_pool(name="pp_conv", bufs=2, space="PSUM") as ppc,
            tc.tile_pool(name="pp_scan", bufs=2, space="PSUM") as pps,
            tc.tile_pool(name="pp_misc", bufs=2, space="PSUM") as ppm,
        ):
            convA = persist.tile([128, NFP * 128], F32R)
            convB = persist.tile([64, NFP * 128], F32R)
            cb = persist.tile([C, 1], F32)
            w_rz = persist.tile([128, 128], F32)
            w_n = persist.tile([H, H], F32)
            w_in_full = persist.tile([128, H], F32)
            w_in = w_in_full[64:128, :]
            b_rz = persist.tile([128, 1], F32)
            b_rz_neg = persist.tile([H, 1], F32)
            b_hn = persist.tile([H, 1], F32)
            b_in = persist.tile([1, H], F32)
            fc_w = persist.tile([H, OUT], F32)
            fc_b = persist.tile([1, OUT], F32)
            ones = persist.tile([1, JW], F32)
            # bigU: rows 0:64 = u_{k-1} at blk k, rows 64:128 = feat_k at blk k
            bigU = persist.tile([128, (t_steps + 1) * nb], F32)
            # bigH: rows 0:64 = h_k at blk k, rows 64:128 = xn_k at blk k
            bigH = persist.tile([128, (t_steps + 1) * nb], F32)
            v_zero = persist.tile([H, nb], F32)

            CW = NFP * 128 // 4
            for ci in range(4):
                cs = slice(ci * CW, (ci + 1) * CW)
                stg = stage.tile([128, CW], F32, tag="stg", name="stg")
                nc.sync.dma_start(out=stg, in_=convA_d[:, cs])
                nc.vector.tensor_copy(convA[:, cs], stg)
            for ci in range(4):
                cs = slice(ci * CW, (ci + 1) * CW)
                stg = stage.tile([128, CW], F32, tag="stg", name="stgb")
                nc.sync.dma_start(out=stg[0:64, :], in_=convB_d[:, cs])
                nc.vector.tensor_copy(convB[:, cs], stg[0:64, :])

            nc.sync.dma_start(out=cb, in_=cb_d[:, :])
            nc.sync.dma_start(out=w_rz, in_=wrz_d[:, :])
            nc.sync.dma_start(out=w_n, in_=wn_d[:, :])
            nc.sync.dma_start(out=w_in, in_=win_d[:, :])
            nc.sync.dma_start(out=b_rz, in_=brz_d[:, :])
            nc.sync.dma_start(out=b_rz_neg, in_=brzn_d[:, :])
            nc.sync.dma_start(out=b_hn, in_=bhn_d[:, :])
            nc.sync.dma_start(out=b_in, in_=bin_d[:, :])
            nc.sync.dma_start(out=fc_w, in_=fcw_d[:, :])
            nc.sync.dma_start(out=fc_b, in_=fcb_d[:, :])
            nc.vector.memset(ones, 1.0)
            nc.vector.memset(bigU[0:64, 0:nb], 0.0)   # u_{-1} = 0
            nc.vector.memset(bigH[0:64, 0:nb], 0.0)   # h_0 = 0
            nc.vector.memset(v_zero, 0.0)             # v_{-1} = 0
            if not full:
                nc.vector.memset(bigU[:, :], 0.0)
                nc.vector.memset(bigH[:, :], 0.0)

            # ---------- X2R staging (persistent, per batch) ----------
            X2Rs = []
            if "conv" in phases:
                for b in range(nb):
                    X2 = x2p.tile([128, t_steps + 2], F32, tag="x2", name="x2")
                    nc.sync.dma_start(out=X2[0:64, 1 : t_steps + 1], in_=x_d[b, :, :])
                    nc.sync.dma_start(out=X2[64:128, 0:t_steps], in_=x_d[b, :, :])
                    nc.vector.memset(X2[0:64, 0:1], 0.0)
                    nc.vector.memset(X2[0:64, t_steps + 1 : t_steps + 2], 0.0)
                    nc.vector.memset(X2[64:128, t_steps : t_steps + 2], 0.0)
                    X2R = persist.tile([128, t_steps + 2], F32R, name=f"x2r{b}")
                    nc.vector.tensor_copy(X2R, X2)
                    X2Rs.append(X2R)

            # ---------- emission units ----------
            conv_state = {}

            def conv_mm(b, th, fp):
                ps = ppc.tile([128, THW], F32, tag="cps", name="cps")
                X2R = X2Rs[b]
                nc.tensor.matmul(
                    ps, convA[:, fp * 128 : (fp + 1) * 128],
                    X2R[:, th * THW : th * THW + THW],
                    start=True, stop=False,
                )
                nc.tensor.matmul(
                    ps, convB[:, fp * 128 : (fp + 1) * 128],
                    X2R[0:64, th * THW + 2 : th * THW + THW + 2],
                    start=False, stop=True,
                )
                if fp == 0:
                    macc = work.tile([128, THW], F32, tag="macc", name="macc")
                    conv_state[(b, th)] = macc
                    nc.vector.tensor_copy(macc, ps)
                else:
                    nc.vector.tensor_max(conv_state[(b, th)],
                                         conv_state[(b, th)], ps)

            def conv_tail(b, th):
                macc = conv_state.pop((b, th))
                mhi = work.tile([64, THW], F32, tag="mhi", name="mhi")
                nc.vector.tensor_copy(mhi, macc[64:128, :])
                m2 = work.tile([64, THW], F32, tag="m2", name="m2")
                nc.vector.tensor_max(m2, macc[0:64, :], mhi)
                out_ap = bigU[64:128, th * THW * nb + b : (th * THW + THW) * nb : nb]
                nc.scalar.activation(out_ap, m2, AF.Relu, bias=cb)

            def xn_unit(j):
                ps = ppm.tile([H, JW], F32, tag="mps", name="xnps")
                nc.tensor.matmul(
                    ps, w_in, bigU[64:128, j * JW : (j + 1) * JW],
                    start=True, stop=False,
                )
                nc.tensor.matmul(ps, b_in, ones, start=False, stop=True)
                nc.scalar.copy(bigH[64:128, j * JW : (j + 1) * JW], ps)

            def fc_unit(j):
                ps = ppm.tile([OUT, JW], F32, tag="mps", name="fcps")
                nc.tensor.matmul(
                    ps, fc_w, bigH[0:64, nb + j * JW : nb + (j + 1) * JW],
                    start=True, stop=False,
                )
                nc.tensor.matmul(ps, fc_b, ones, start=False, stop=True)
                ob = work.tile([OUT, JW], F32, tag="ob", name="ob")
                nc.scalar.copy(ob, ps)
                tpj = JW // nb
                for b in range(nb):
                    nc.sync.dma_start(
                        out=out_d[b, 0:OUT, j * tpj : (j + 1) * tpj],
                        in_=ob[:, b : JW : nb],
                    )

            def scan_step(k, prev_v):
                col = slice(k * nb, (k + 1) * nb)
                ncol = slice((k + 1) * nb, (k + 2) * nb)
                psum_rz = pps.tile([128, nb], F32, tag="rz", name="rz")
                psum_hn = pps.tile([H, nb], F32, tag="hn", name="hn")
                nc.tensor.matmul(psum_rz, w_rz, bigU[:, col], start=True, stop=False)
                nc.tensor.matmul(psum_hn, w_n, bigH[0:64, col], start=True, stop=True)
                nc.tensor.matmul(psum_rz, w_rz[0:64, :], prev_v, start=False, stop=True)

                r_s = scanw.tile([H, nb], F32, tag="rs", name="rs")
                nc.scalar.activation(r_s, psum_rz[0:64, :], AF.Sigmoid,
                                     bias=b_rz[0:64, :])
                z_s = scanw.tile([H, nb], F32, tag="zs", name="zs")
                nc.scalar.activation(z_s, psum_rz[64:128, :], AF.Sigmoid,
                                     bias=b_rz[64:128, :])
                zb_s = scanw.tile([H, nb], F32, tag="zbs", name="zbs")
                nc.scalar.activation(zb_s, psum_rz[64:128, :], AF.Sigmoid,
                                     bias=b_rz_neg, scale=-1.0)
                nc.vector.tensor_mul(bigU[0:64, ncol], z_s, bigH[0:64, col])
                q = scanw.tile([128, nb], F32, tag="q", name="q")
                nc.vector.scalar_tensor_tensor(
                    out=q[64:128, :], in0=psum_hn, scalar=b_hn, in1=r_s,
                    op0=OP.add, op1=OP.mult,
                )
                q2 = scanw.tile([H, nb], F32, tag="q2", name="q2")
                nc.vector.tensor_add(q2, q[64:128, :], bigH[64:128, col])
                n_t = scanw.tile([H, nb], F32, tag="n", name="n")
                nc.scalar.activation(n_t, q2, AF.Tanh)
                v_t = scanw.tile([H, nb], F32, tag="v", name="v")
                nc.vector.tensor_mul(v_t, zb_s, n_t)
                nc.vector.tensor_add(bigH[0:64, ncol], bigU[0:64, ncol], v_t)
                return v_t

            rep_ctx = tc.For_i(0, reps, 1) if reps > 1 else contextlib.nullcontext()
            with rep_ctx:
                if not inter:
                    for b in range(nb if "conv" in phases else 0):
                        for th in range(NTH):
                            for fp in range(NFP):
                                conv_mm(b, th, fp)
                            conv_tail(b, th)
                    for j in range(NJ if "xn" in phases else 0):
                        xn_unit(j)
                    prev_v = v_zero
                    for k in range(t_steps if "scan" in phases else 0):
                        prev_v = scan_step(k, prev_v)
                    for j in range(NJ if "fc" in phases else 0):
                        fc_unit(j)
                else:
                    # th=0 conv upfront + first-half xn
                    for b in range(nb):
                        for fp in range(NFP):
                            conv_mm(b, 0, fp)
                        conv_tail(b, 0)
                    for j in range(NJ // 2):
                        xn_unit(j)

                    # conv th=1 spread over scan steps [8, 440); 2nd-half xn
                    # after it; each fc tile as soon as its h-range is done.
                    units = []
                    for b in range(nb):
                        for fp in range(NFP):
                            units.append(("mm", b, fp))
                        units.append(("tail", b))
                    sched = {}
                    lo, hi = 8, 440
                    for i, u in enumerate(units):
                        k_at = lo + (i * (hi - lo)) // len(units)
                        sched.setdefault(k_at, []).append(u)
                    for j in range(NJ // 2, NJ):
                        sched.setdefault(444 + 8 * (j - NJ // 2), []).append(("xn", j))
                    tpj = JW // nb
                    for j in range(NJ):
                        k_at = (j + 1) * tpj
                        if k_at < t_steps:
                            sched.setdefault(k_at, []).append(("fc", j))

                    prev_v = v_zero
                    for k in range(t_steps):
                        prev_v = scan_step(k, prev_v)
                        for u in sched.get(k, ()):
                            if u[0] == "mm":
                                conv_mm(u[1], 1, u[2])
                            elif u[0] == "tail":
                                conv_tail(u[1], 1)
                            elif u[0] == "xn":
                                xn_unit(u[1])
                            elif u[0] == "fc":
                                fc_unit(u[1])
                    for j in range(NJ):
                        if (j + 1) * tpj >= t_steps:
                            fc_unit(j)

    nc.finalize()
    return nc


def prep_weights(conv_w, conv_b, w_ih, w_hh, b_ih, b_hh, fc_w, fc_b):
    """Host-side rearrangement of the small weights into device layouts."""
    conv_w = np.asarray(conv_w, np.float32)
    A = np.zeros((128, NFP * 128), np.float32)
    Bm = np.zeros((64, NFP * 128), np.float32)
    for fp in range(NFP):
        for fo in range(2):
            fout = 2 * fp + fo
            for fprime in range(max(0, fout - 1), min(64, fout + 2)):
                i = fprime - fout + 1
                cols = slice(fp * 128 + fo * 64, fp * 128 + fo * 64 + 64)
                A[fprime, cols] = conv_w[:, 0, i, 0]
                A[64 + fprime, cols] = conv_w[:, 0, i, 1]
                Bm[fprime, cols] = conv_w[:, 0, i, 2]
    w_ih = np.asarray(w_ih, np.float32)
    w_hh = np.asarray(w_hh, np.float32)
    b_ih = np.asarray(b_ih, np.float32)
    b_hh = np.asarray(b_hh, np.float32)
    return {
        "convA": A,
        "convB": Bm,
        "conv_bias": np.asarray(conv_b, np.float32).reshape(C, 1),
        "w_rz_lhsT": np.concatenate(
            [w_hh[0:128, :].T, w_ih[0:128, :].T], axis=0
        ).astype(np.float32).copy(),
        "w_n_lhsT": w_hh[128:192, :].T.astype(np.float32).copy(),
        "w_in_lhsT": w_ih[128:192, :].T.astype(np.float32).copy(),
        "b_rz": (b_ih[0:128] + b_hh[0:128]).reshape(128, 1).astype(np.float32),
        "b_rz_neg": (-(b_ih[64:128] + b_hh[64:128])).reshape(H, 1).astype(np.float32),
        "b_hn": b_hh[128:192].reshape(H, 1).astype(np.float32),
        "b_in_row": b_ih[128:192].reshape(1, H).astype(np.float32),
        "fc_lhsT": np.asarray(fc_w, np.float32).T.copy(),
        "fc_b_row": np.asarray(fc_b, np.float32).reshape(1, OUT),
    }


_NC_CACHE = {}


def _get_nc():
    if "nc" not in _NC_CACHE:
        _NC_CACHE["nc"] = build_crnn()
    return _NC_CACHE["nc"]


def run(inputs, trace=False):
    """Returns (out [B, OUT, T], BassKernelResults)."""
    x = np.asarray(inputs["x"], np.float32)
    wd = prep_weights(
        inputs["conv_w"], inputs["conv_b"], inputs["w_ih"], inputs["w_hh"],
        inputs["b_ih"], inputs["b_hh"], inputs["fc_w"], inputs["fc_b"],
    )
    nc = _get_nc()
    in_maps = []
    for i in range(NCORES):
        m = dict(wd)
        m["x"] = np.ascontiguousarray(x[i * NB : (i + 1) * NB])
        in_maps.append(m)
    res = run_bass_kernel_spmd(nc, in_maps, list(range(NCORES)), trace=trace)
    out = np.concatenate([res.results[i]["out"] for i in range(NCORES)], axis=0)
    return out, res


def kernel(**inputs) -> np.ndarray:
    out, _ = run(inputs, trace=False)
    return out



# revision 6
# speedup vs baseline: 24.8392x; 24.8392x over previous
"""CRNN v2: restructured for shorter scan critical cycle + cleaner overlap.

Per core (nb=8 batch items):
  - conv in 256-col time quarters; quarter 0 is the prologue, quarters 1-3
    are interleaved into the scan's idle engine slots. relu+bias folded into
    DVE tensor_scalar ops (ACT is reserved for the scan's sigmoid/tanh).
  - GRU scan with w = -v decomposition: h_{k+1} = u_{k+1} - w_k,
    u_{k+1} = z_k*h_k, w_k = (z_k-1)*n_k. Sign-folded weights let the rz and
    hn matmuls consume (u, w) directly, so no h assembly on the cycle.
    One merged sigmoid over [128,nb] yields r and z; zb is gone.
    q/q2 run on the Pool engine (gpsimd), w on DVE, u/h' on Pool off-cycle.
"""

import contextlib
import numpy as np

import concourse.bass as bass
import concourse.mybir as mybir
import concourse.tile as tile
from concourse import bacc
from concourse.bass_utils import run_bass_kernel_spmd

F32 = mybir.dt.float32
F32R = mybir.dt.float32r
AF = mybir.ActivationFunctionType
OP = mybir.AluOpType

B, F, T = 64, 64, 1024
C = 64
H = 64
OUT = 2
NCORES = 8
NB = B // NCORES
NFP = F // 2


def build_crnn(nb=NB, t_steps=T, reps=1, phases=("conv", "xn", "scan", "fc"),
               interleave=True, opts=None):
    opts = dict(opts or {})
    q_engine = opts.get("q_engine", "dve")        # dve (Pool has no tensor ops on this ISA)
    uh_engine = opts.get("uh_engine", "dve")      # dve only
    g_pre = opts.get("g_pre", True)               # materialize g=psum_hn+b_hn (DVE; Pool cannot read PSUM)
    thw = opts.get("thw", 256)
    mm_order = opts.get("mm_order", "rz_first")   # rz_first | hn_first
    scan_split = opts.get("scan_split", False)    # unused (Pool has no tensor ops)
    conv_dve_split = opts.get("conv_dve_split", True)   # 128-col conv maxes in scan region
    xn_on_act = opts.get("xn_on_act", True)       # xn/fc psum->sbuf copies on ACT
    g_on_act = opts.get("g_on_act", False)        # g = psum_hn + b_hn on ACT
    pps_merge = opts.get("pps_merge", False)      # rz+hn share one PSUM bank
    burst2 = opts.get("burst2", False)            # conv units paired on even slots

    nc = bacc.Bacc("TRN2", target_bir_lowering=False, debug=False)
    TB = t_steps * nb
    NTH = max(1, t_steps // thw)
    THW = min(thw, t_steps)
    NJ = max(1, TB // 512)
    JW = min(512, TB)
    full = len(phases) == 4
    inter = interleave and full and t_steps == T

    x_d = nc.declare_dram_parameter("x", [nb, F, t_steps], F32R, isOutput=False)
    convA_d = nc.declare_dram_parameter("convA", [128, NFP * 128], F32R, isOutput=False)
    convB_d = nc.declare_dram_parameter("convB", [64, NFP * 128], F32R, isOutput=False)
    cb_d = nc.declare_dram_parameter("conv_bias", [C, 1], F32, isOutput=False)
    wrz_d = nc.declare_dram_parameter("w_rz_lhsT", [128, 128], F32, isOutput=False)
    wrzvn_d = nc.declare_dram_parameter("w_rzv_neg", [H, 128], F32, isOutput=False)
    wn_d = nc.declare_dram_parameter("w_n_lhsT", [H, H], F32, isOutput=False)
    wnn_d = nc.declare_dram_parameter("w_n_neg", [H, H], F32, isOutput=False)
    win_d = nc.declare_dram_parameter("w_in_lhsT", [C, H], F32, isOutput=False)
    brz_d = nc.declare_dram_parameter("b_rz", [128, 1], F32, isOutput=False)
    bhn_d = nc.declare_dram_parameter("b_hn", [H, 1], F32, isOutput=False)
    bin_d = nc.declare_dram_parameter("b_in_row", [1, H], F32, isOutput=False)
    fcw_d = nc.declare_dram_parameter("fc_lhsT", [H, OUT], F32, isOutput=False)
    fcb_d = nc.declare_dram_parameter("fc_b_row", [1, OUT], F32, isOutput=False)
    out_d = nc.declare_dram_parameter("out", [nb, OUT, t_steps], F32, isOutput=True)

    with tile.TileContext(nc) as tc:
        with (
            tc.tile_pool(name="persist", bufs=1) as persist,
            tc.tile_pool(name="stage", bufs=2) as stage,
            tc.tile_pool(name="x2pool", bufs=1) as x2p,
            tc.tile_pool(name="work", bufs=3) as work,
            tc.tile_pool(name="scanw", bufs=3) as scanw,
            tc.tile_pool(name="pp_conv", bufs=(5 if pps_merge else 3),
                         space="PSUM") as ppc,
            tc.tile_pool(name="pp_scan", bufs=2, space="PSUM") as pps,
            tc.tile_pool(name="pp_misc", bufs=1, space="PSUM") as ppm,
        ):
            convA = persist.tile([128, NFP * 128], F32R)
            convB = persist.tile([64, NFP * 128], F32R)
            cb = persist.tile([C, 1], F32)
            w_rz = persist.tile([128, 128], F32)
            w_rzv_neg = persist.tile([H, 128], F32)
            w_n = persist.tile([H, H], F32)
            w_n_neg = persist.tile([H, H], F32)
            w_in_full = persist.tile([128, H], F32)
            w_in = w_in_full[64:128, :]
            b_rz = persist.tile([128, 1], F32)
            b_hn = persist.tile([H, 1], F32)
            b_in = persist.tile([1, H], F32)
            fc_w = persist.tile([H, OUT], F32)
            fc_b = persist.tile([1, OUT], F32)
            ones = persist.tile([1, JW], F32)
            # bigU: rows 0:64 = u_k at blk k, rows 64:128 = feat_k at blk k
            bigU = persist.tile([128, (t_steps + 1) * nb], F32)
            # bigH: h_k at blk k (partitions 0:64); bigX: xn_k at blk k
            bigH = persist.tile([64, (t_steps + 1) * nb], F32)
            bigX = persist.tile([64, (t_steps + 1) * nb], F32)
            w_zero = persist.tile([H, nb], F32)

            # direct f32r DMAs: same bit layout as f32, no staging copies
            nc.sync.dma_start(out=convA, in_=convA_d[:, :])
            nc.sync.dma_start(out=convB, in_=convB_d[:, :])
            X2Rs = []
            if "conv" in phases:
                for b in range(nb):
                    X2R = persist.tile([128, t_steps + 2], F32R, name=f"x2r{b}")
                    nc.sync.dma_start(out=X2R[0:64, 1 : t_steps + 1], in_=x_d[b, :, :])
                    nc.sync.dma_start(out=X2R[64:128, 0:t_steps], in_=x_d[b, :, :])
                    nc.vector.memset(X2R[0:64, 0:1].bitcast(F32), 0.0)
                    nc.vector.memset(
                        X2R[0:64, t_steps + 1 : t_steps + 2].bitcast(F32), 0.0)
                    nc.vector.memset(
                        X2R[64:128, t_steps : t_steps + 2].bitcast(F32), 0.0)
                    X2Rs.append(X2R)

            nc.sync.dma_start(out=cb, in_=cb_d[:, :])
            nc.sync.dma_start(out=w_rz, in_=wrz_d[:, :])
            nc.sync.dma_start(out=w_rzv_neg, in_=wrzvn_d[:, :])
            nc.sync.dma_start(out=w_n, in_=wn_d[:, :])
            nc.sync.dma_start(out=w_n_neg, in_=wnn_d[:, :])
            nc.sync.dma_start(out=w_in, in_=win_d[:, :])
            nc.sync.dma_start(out=b_rz, in_=brz_d[:, :])
            nc.sync.dma_start(out=b_hn, in_=bhn_d[:, :])
            nc.sync.dma_start(out=b_in, in_=bin_d[:, :])
            nc.sync.dma_start(out=fc_w, in_=fcw_d[:, :])
            nc.sync.dma_start(out=fc_b, in_=fcb_d[:, :])
            nc.vector.memset(ones, 1.0)
            nc.vector.memset(bigU[0:64, 0:nb], 0.0)   # u_0 = 0
            nc.vector.memset(bigH[:, 0:nb], 0.0)      # h_0 = 0
            nc.vector.memset(w_zero, 0.0)             # w_{-1} = 0
            if not full:
                nc.vector.memset(bigU[:, :], 0.0)
                nc.vector.memset(bigH[:, :], 0.0)
                nc.vector.memset(bigX[:, :], 0.0)

            # ---------- emission units ----------
            conv_state = {}

            def conv_mm(b, th, fp, split=False):
                ps = ppc.tile([128, THW], F32, tag="cps", name="cps")
                X2R = X2Rs[b]
                nc.tensor.matmul(
                    ps, convA[:, fp * 128 : (fp + 1) * 128],
                    X2R[:, th * THW : th * THW + THW],
                    start=True, stop=False,
                )
                nc.tensor.matmul(
                    ps, convB[:, fp * 128 : (fp + 1) * 128],
                    X2R[0:64, th * THW + 2 : th * THW + THW + 2],
                    start=False, stop=True,
                )
                st = conv_state.setdefault((b, th), [None, None])
                halves = ((0, THW // 2), (THW // 2, THW)) if split else ((0, THW),)
                if st[0] is None:
                    macc = work.tile([128, THW], F32, tag=f"macc{b % 4}",
                                     name="macc0")
                    st[0] = macc
                    for lo, hi in halves:
                        nc.vector.tensor_copy(macc[:, lo:hi], ps[:, lo:hi])
                else:
                    for lo, hi in halves:
                        nc.vector.tensor_max(st[0][:, lo:hi], st[0][:, lo:hi],
                                             ps[:, lo:hi])

            def conv_tail(b, th):
                maccD, maccP = conv_state.pop((b, th))
                if maccP is not None:
                    nc.vector.tensor_max(maccD, maccD, maccP)
                # relu(max(lo,hi)+cb) == max(relu(lo+cb), relu(hi+cb))
                t1 = work.tile([64, THW], F32, tag="t1", name="t1")
                nc.vector.tensor_scalar(t1, maccD[0:64, :], cb, 0.0, OP.add, OP.max)
                thi = work.tile([64, THW], F32, tag="thi", name="thi")
                nc.vector.tensor_copy(thi, maccD[64:128, :])
                t2 = work.tile([64, THW], F32, tag="t2", name="t2")
                nc.vector.tensor_scalar(t2, thi, cb, 0.0, OP.add, OP.max)
                out_ap = bigU[64:128, th * THW * nb + b : (th * THW + THW) * nb : nb]
                nc.vector.scalar_tensor_tensor(
                    out=out_ap, in0=t1, scalar=0.0, in1=t2,
                    op0=OP.add, op1=OP.max,
                )

            def xn_unit(j, half=None):
                ps = ppm.tile([H, JW], F32, tag="mps", name="xnps")
                nc.tensor.matmul(
                    ps, w_in, bigU[64:128, j * JW : (j + 1) * JW],
                    start=True, stop=False,
                )
                nc.tensor.matmul(ps, b_in, ones, start=False, stop=True)
                hw = JW // 2
                ce = nc.scalar.copy if xn_on_act else nc.vector.tensor_copy
                ce(bigX[:, j * JW : j * JW + hw], ps[:, 0:hw])
                ce(bigX[:, j * JW + hw : (j + 1) * JW], ps[:, hw:JW])

            def fc_unit(j):
                ps = ppm.tile([OUT, JW], F32, tag="mps", name="fcps")
                nc.tensor.matmul(
                    ps, fc_w, bigH[:, nb + j * JW : nb + (j + 1) * JW],
                    start=True, stop=False,
                )
                nc.tensor.matmul(ps, fc_b, ones, start=False, stop=True)
                ob = work.tile([OUT, JW], F32, tag="ob", name="ob")
                if xn_on_act:
                    nc.scalar.copy(ob[:, 0 : JW // 2], ps[:, 0 : JW // 2])
                    nc.scalar.copy(ob[:, JW // 2 : JW], ps[:, JW // 2 : JW])
                else:
                    nc.vector.tensor_copy(ob, ps)
                tpj = JW // nb
                for b in range(nb):
                    nc.sync.dma_start(
                        out=out_d[b, 0:OUT, j * tpj : (j + 1) * tpj],
                        in_=ob[:, b : JW : nb],
                    )

            qe = nc.gpsimd if q_engine == "pool" else nc.vector
            ue = nc.gpsimd if uh_engine == "pool" else nc.vector

            def scan_step(k, prev_w):
                col = slice(k * nb, (k + 1) * nb)
                ncol = slice((k + 1) * nb, (k + 2) * nb)
                if pps_merge:
                    psum_sc = pps.tile([128, 2 * nb], F32, tag="sc", name="sc")
                    psum_rz = psum_sc[:, 0:nb]
                    psum_hn = psum_sc[0:64, nb : 2 * nb]
                else:
                    psum_rz = pps.tile([128, nb], F32, tag="rz", name="rz")
                    psum_hn = pps.tile([H, nb], F32, tag="hn", name="hn")
                # PE: all four step-k matmuls; operands (u_k, w_{k-1}) were
                # produced ~a full period ago, so these run back-to-back as
                # soon as the PSUM banks free up.
                if mm_order == "rz_first":
                    nc.tensor.matmul(psum_rz, w_rz, bigU[:, col], start=True, stop=False)
                    nc.tensor.matmul(psum_rz, w_rzv_neg, prev_w, start=False, stop=True)
                    nc.tensor.matmul(psum_hn, w_n, bigU[0:64, col], start=True, stop=False)
                    nc.tensor.matmul(psum_hn, w_n_neg, prev_w, start=False, stop=True)
                else:
                    nc.tensor.matmul(psum_hn, w_n, bigU[0:64, col], start=True, stop=False)
                    nc.tensor.matmul(psum_hn, w_n_neg, prev_w, start=False, stop=True)
                    nc.tensor.matmul(psum_rz, w_rz, bigU[:, col], start=True, stop=False)
                    nc.tensor.matmul(psum_rz, w_rzv_neg, prev_w, start=False, stop=True)

                # ACT: r first (critical), then z; both land at partition 0
                r_s = scanw.tile([H, nb], F32, tag="rs", name="rs")
                nc.scalar.activation(r_s, psum_rz[0:64, :], AF.Sigmoid,
                                     bias=b_rz[0:64, :])
                z_s = scanw.tile([H, nb], F32, tag="zs", name="zs")
                nc.scalar.activation(z_s, psum_rz[64:128, :], AF.Sigmoid,
                                     bias=b_rz[64:128, :])

                # Pool: q = (psum_hn + b_hn) * r ; t = q + xn
                if g_pre:
                    g_t = scanw.tile([H, nb], F32, tag="g", name="g")
                    if g_on_act:
                        nc.scalar.activation(g_t, psum_hn, AF.Identity, bias=b_hn)
                    else:
                        nc.vector.tensor_scalar(g_t, psum_hn, b_hn, 0.0,
                                                OP.add, OP.add)
                    q_t = scanw.tile([H, nb], F32, tag="q", name="q")
                    qe.tensor_mul(q_t, g_t, r_s)
                else:
                    q_t = scanw.tile([H, nb], F32, tag="q", name="q")
                    qe.scalar_tensor_tensor(
                        out=q_t, in0=psum_hn, scalar=b_hn, in1=r_s,
                        op0=OP.add, op1=OP.mult,
                    )
                t_t = scanw.tile([H, nb], F32, tag="t", name="t")
                qe.tensor_add(t_t, q_t, bigX[:, col])

                # ACT: tanh
                n_t = scanw.tile([H, nb], F32, tag="n", name="n")
                nc.scalar.activation(n_t, t_t, AF.Tanh)

                # DVE: w = (z - 1) * n   (= -v, the only late operand).
                # Kept alone at the head of DVE program order so conv work
                # behind it can never delay the critical chain.
                w_t = scanw.tile([H, nb], F32, tag="w", name="w")
                nc.vector.scalar_tensor_tensor(
                    out=w_t, in0=z_s, scalar=1.0, in1=n_t,
                    op0=OP.subtract, op1=OP.mult,
                )
                # off-cycle: u_{k+1} = z*h_k ; h_{k+1} = u_{k+1} - w_k
                ue.tensor_mul(bigU[0:64, ncol], z_s, bigH[:, col])
                ue.tensor_sub(bigH[:, ncol], bigU[0:64, ncol], w_t)
                return w_t

            rep_ctx = tc.For_i(0, reps, 1) if reps > 1 else contextlib.nullcontext()
            with rep_ctx:
                if not inter:
                    for b in range(nb if "conv" in phases else 0):
                        for th in range(NTH):
                            for fp in range(NFP):
                                conv_mm(b, th, fp)
                            conv_tail(b, th)
                    for j in range(NJ if "xn" in phases else 0):
                        xn_unit(j)
                    prev_w = w_zero
                    for k in range(t_steps if "scan" in phases else 0):
                        prev_w = scan_step(k, prev_w)
                    for j in range(NJ if "fc" in phases else 0):
                        fc_unit(j)
                else:
                    # prologue: conv quarter 0, four batches interleaved so the
                    # DVE max chains always have ready work
                    qd = opts.get("pro_quads", 2)
                    for b0 in range(0, nb, qd):
                        for fp in range(NFP):
                            for bo in range(qd):
                                conv_mm(b0 + bo, 0, fp, split=False)
                        for bo in range(qd):
                            conv_tail(b0 + bo, 0)
                    for j in range(4):
                        xn_unit(j)

                    # interleave schedule:
                    #  conv quarter q over steps [start_q, end_q), xn units for
                    #  t-range right after their conv quarter completes, fc as
                    #  h-ranges finish.
                    sched = {}

                    def spread(units, lo, hi):
                        for i, u in enumerate(units):
                            k_at = lo + (i * (hi - lo)) // len(units)
                            if burst2:
                                k_at -= k_at % 2
                            sched.setdefault(k_at, []).append(u)

                    for th, (lo, hi) in ((1, (1, 248)), (2, (260, 504)),
                                         (3, (516, 760))):
                        units = []
                        for b in range(nb):
                            for fp in range(NFP):
                                units.append(("mm", b, th, fp))
                            units.append(("tail", b, th))
                        spread(units, lo, hi)
                    for j in range(4, 8):
                        sched.setdefault(250 + 2 * (j - 4), []).append(("xn", j))
                    for j in range(8, 12):
                        sched.setdefault(506 + 2 * (j - 8), []).append(("xn", j))
                    for j in range(12, 16):
                        sched.setdefault(762 + 2 * (j - 12), []).append(("xn", j))
                    tpj = JW // nb
                    for j in range(NJ):
                        # fc can run any time after its h-range exists; keep it
                        # out of the conv-congested first ~760 steps
                        k_at = max((j + 1) * tpj + 4, 772 + j * 14)
                        if k_at < t_steps:
                            sched.setdefault(k_at, []).append(("fc", j))

                    prev_w = w_zero
                    for k in range(t_steps):
                        prev_w = scan_step(k, prev_w)
                        for u in sched.get(k, ()):
                            if u[0] == "mm":
                                conv_mm(u[1], u[2], u[3], split=conv_dve_split)
                            elif u[0] == "tail":
                                conv_tail(u[1], u[2])
                            elif u[0] == "xn":
                                xn_unit(u[1])
                            elif u[0] == "fc":
                                fc_unit(u[1])
                    for j in range(NJ):
                        if (j + 1) * tpj + 4 >= t_steps:
                            fc_unit(j)

    nc.finalize()
    return nc


def prep_weights(conv_w, conv_b, w_ih, w_hh, b_ih, b_hh, fc_w, fc_b):
    """Host-side rearrangement of the small weights into device layouts."""
    conv_w = np.asarray(conv_w, np.float32)
    A = np.zeros((128, NFP * 128), np.float32)
    Bm = np.zeros((64, NFP * 128), np.float32)
    for fp in range(NFP):
        for fo in range(2):
            fout = 2 * fp + fo
            for fprime in range(max(0, fout - 1), min(64, fout + 2)):
                i = fprime - fout + 1
                cols = slice(fp * 128 + fo * 64, fp * 128 + fo * 64 + 64)
                A[fprime, cols] = conv_w[:, 0, i, 0]
                A[64 + fprime, cols] = conv_w[:, 0, i, 1]
                Bm[fprime, cols] = conv_w[:, 0, i, 2]
    w_ih = np.asarray(w_ih, np.float32)
    w_hh = np.asarray(w_hh, np.float32)
    b_ih = np.asarray(b_ih, np.float32)
    b_hh = np.asarray(b_hh, np.float32)
    w_rz_h = w_hh[0:128, :].T.astype(np.float32)       # [64,128]
    w_rz_x = w_ih[0:128, :].T.astype(np.float32)       # [64,128]
    return {
        "convA": A,
        "convB": Bm,
        "conv_bias": np.asarray(conv_b, np.float32).reshape(C, 1),
        "w_rz_lhsT": np.concatenate([w_rz_h, w_rz_x], axis=0).copy(),
        "w_rzv_neg": (-w_rz_h).copy(),
        "w_n_lhsT": w_hh[128:192, :].T.astype(np.float32).copy(),
        "w_n_neg": (-w_hh[128:192, :].T).astype(np.float32).copy(),
        "w_in_lhsT": w_ih[128:192, :].T.astype(np.float32).copy(),
        "b_rz": (b_ih[0:128] + b_hh[0:128]).reshape(128, 1).astype(np.float32),
        "b_hn": b_hh[128:192].reshape(H, 1).astype(np.float32),
        "b_in_row": b_ih[128:192].reshape(1, H).astype(np.float32),
        "fc_lhsT": np.asarray(fc_w, np.float32).T.copy(),
        "fc_b_row": np.asarray(fc_b, np.float32).reshape(1, OUT),
    }


_NC_CACHE = {}


def _get_nc():
    if "nc" not in _NC_CACHE:
        _NC_CACHE["nc"] = build_crnn()
    return _NC_CACHE["nc"]


def run(inputs, trace=False):
    """Returns (out [B, OUT, T], BassKernelResults)."""
    x = np.asarray(inputs["x"], np.float32)
    wd = prep_weights(
        inputs["conv_w"], inputs["conv_b"], inputs["w_ih"], inputs["w_hh"],
        inputs["b_ih"], inputs["b_hh"], inputs["fc_w"], inputs["fc_b"],
    )
    nc = _get_nc()
    in_maps = []
    for i in range(NCORES):
        m = dict(wd)
        m["x"] = np.ascontiguousarray(x[i * NB : (i + 1) * NB])
        in_maps.append(m)
    res = run_bass_kernel_spmd(nc, in_maps, list(range(NCORES)), trace=trace)
    out = np.concatenate([res.results[i]["out"] for i in range(NCORES)], axis=0)
    return out, res


def kernel(**inputs) -> np.ndarray:
    out, _ = run(inputs, trace=False)
    return out


# revision 7
# speedup vs baseline: 34.6580x; 1.3953x over previous
"""CRNN (conv3x3 -> ReLU -> freq-maxpool -> GRU scan -> FC) on 8 Trainium2
NeuronCores, data-parallel over batch (8 items per core).

Structure per core:
  - conv: banded-weight matmuls over the frequency contraction; time shifts
    via column offsets into a padded fp32r tile; two accumulating matmuls per
    f-pair give PSUM [128 = 2f x 64c, 512t]; running tensor_max over f-pairs
    + ReLU(+bias) writes feat[c, t] batch-interleaved into bigU[64:128].
  - xn = W_ihn @ feat + b_ihn precomputed (PE), packed into bigH[64:128].
  - GRU scan with u/v decomposition: h_{k+1} = u_k + v_k, u_k = z_k*h_k,
    v_k = (1-z_k)*n_k. The rz matmul takes [u; feat] (K=128) plus a separate
    v matmul (K=64), so the only late operand on the serial chain is v.
  - FC from bigH h-history, output DMA'd straight from PSUM.
  - The time-half-1 conv work, the second half of xn, and the FC tiles are
    emitted interleaved with the scan steps so they execute in the scan's
    idle engine slots.
"""

import contextlib
import numpy as np

import concourse.bass as bass
import concourse.mybir as mybir
import concourse.tile as tile
from concourse import bacc
from concourse.bass_utils import run_bass_kernel_spmd

F32 = mybir.dt.float32
F32R = mybir.dt.float32r
AF = mybir.ActivationFunctionType
OP = mybir.AluOpType

B, F, T = 64, 64, 1024
C = 64
H = 64
OUT = 2
NCORES = 8
NB = B // NCORES
NFP = F // 2


def build_crnn(nb=NB, t_steps=T, reps=1, phases=("conv", "xn", "scan", "fc"),
               interleave=True):
    nc = bacc.Bacc("TRN2", target_bir_lowering=False, debug=False)
    TB = t_steps * nb
    NTH = max(1, t_steps // 512)
    THW = min(512, t_steps)
    NJ = max(1, TB // 512)
    JW = min(512, TB)
    full = len(phases) == 4
    inter = interleave and full and t_steps == T

    x_d = nc.declare_dram_parameter("x", [nb, F, t_steps], F32, isOutput=False)
    convA_d = nc.declare_dram_parameter("convA", [128, NFP * 128], F32, isOutput=False)
    convB_d = nc.declare_dram_parameter("convB", [64, NFP * 128], F32, isOutput=False)
    cb_d = nc.declare_dram_parameter("conv_bias", [C, 1], F32, isOutput=False)
    wrz_d = nc.declare_dram_parameter("w_rz_lhsT", [128, 128], F32, isOutput=False)
    wn_d = nc.declare_dram_parameter("w_n_lhsT", [H, H], F32, isOutput=False)
    win_d = nc.declare_dram_parameter("w_in_lhsT", [C, H], F32, isOutput=False)
    brz_d = nc.declare_dram_parameter("b_rz", [128, 1], F32, isOutput=False)
    brzn_d = nc.declare_dram_parameter("b_rz_neg", [H, 1], F32, isOutput=False)
    bhn_d = nc.declare_dram_parameter("b_hn", [H, 1], F32, isOutput=False)
    bin_d = nc.declare_dram_parameter("b_in_row", [1, H], F32, isOutput=False)
    fcw_d = nc.declare_dram_parameter("fc_lhsT", [H, OUT], F32, isOutput=False)
    fcb_d = nc.declare_dram_parameter("fc_b_row", [1, OUT], F32, isOutput=False)
    out_d = nc.declare_dram_parameter("out", [nb, OUT, t_steps], F32, isOutput=True)

    with tile.TileContext(nc) as tc:
        with (
            tc.tile_pool(name="persist", bufs=1) as persist,
            tc.tile_pool(name="stage", bufs=2) as stage,
            tc.tile_pool(name="x2pool", bufs=1) as x2p,
            tc.tile_pool(name="work", bufs=2) as work,
            tc.tile_pool(name="scanw", bufs=3) as scanw,
            tc.tile_pool(name="pp_conv", bufs=2, space="PSUM") as ppc,
            tc.tile_pool(name="pp_scan", bufs=2, space="PSUM") as pps,
            tc.tile_pool(name="pp_misc", bufs=2, space="PSUM") as ppm,
        ):
            convA = persist.tile([128, NFP * 128], F32R)
            convB = persist.tile([64, NFP * 128], F32R)
            cb = persist.tile([C, 1], F32)
            w_rz = persist.tile([128, 128], F32)
            w_n = persist.tile([H, H], F32)
            w_in_full = persist.tile([128, H], F32)
            w_in = w_in_full[64:128, :]
            b_rz = persist.tile([128, 1], F32)
            b_rz_neg = persist.tile([H, 1], F32)
            b_hn = persist.tile([H, 1], F32)
            b_in = persist.tile([1, H], F32)
            fc_w = persist.tile([H, OUT], F32)
            fc_b = persist.tile([1, OUT], F32)
            ones = persist.tile([1, JW], F32)
            # bigU: rows 0:64 = u_{k-1} at blk k, rows 64:128 = feat_k at blk k
            bigU = persist.tile([128, (t_steps + 1) * nb], F32)
            # bigH: rows 0:64 = h_k at blk k, rows 64:128 = xn_k at blk k
            bigH = persist.tile([128, (t_steps + 1) * nb], F32)
            v_zero = persist.tile([H, nb], F32)

            CW = NFP * 128 // 4
            for ci in range(4):
                cs = slice(ci * CW, (ci + 1) * CW)
                stg = stage.tile([128, CW], F32, tag="stg", name="stg")
                nc.sync.dma_start(out=stg, in_=convA_d[:, cs])
                nc.vector.tensor_copy(convA[:, cs], stg)
            for ci in range(4):
                cs = slice(ci * CW, (ci + 1) * CW)
                stg = stage.tile([128, CW], F32, tag="stg", name="stgb")
                nc.sync.dma_start(out=stg[0:64, :], in_=convB_d[:, cs])
                nc.vector.tensor_copy(convB[:, cs], stg[0:64, :])

            nc.sync.dma_start(out=cb, in_=cb_d[:, :])
            nc.sync.dma_start(out=w_rz, in_=wrz_d[:, :])
            nc.sync.dma_start(out=w_n, in_=wn_d[:, :])
            nc.sync.dma_start(out=w_in, in_=win_d[:, :])
            nc.sync.dma_start(out=b_rz, in_=brz_d[:, :])
            nc.sync.dma_start(out=b_rz_neg, in_=brzn_d[:, :])
            nc.sync.dma_start(out=b_hn, in_=bhn_d[:, :])
            nc.sync.dma_start(out=b_in, in_=bin_d[:, :])
            nc.sync.dma_start(out=fc_w, in_=fcw_d[:, :])
            nc.sync.dma_start(out=fc_b, in_=fcb_d[:, :])
            nc.vector.memset(ones, 1.0)
            nc.vector.memset(bigU[0:64, 0:nb], 0.0)   # u_{-1} = 0
            nc.vector.memset(bigH[0:64, 0:nb], 0.0)   # h_0 = 0
            nc.vector.memset(v_zero, 0.0)             # v_{-1} = 0
            if not full:
                nc.vector.memset(bigU[:, :], 0.0)
                nc.vector.memset(bigH[:, :], 0.0)

            # ---------- X2R staging (persistent, per batch) ----------
            X2Rs = []
            if "conv" in phases:
                for b in range(nb):
                    X2 = x2p.tile([128, t_steps + 2], F32, tag="x2", name="x2")
                    nc.sync.dma_start(out=X2[0:64, 1 : t_steps + 1], in_=x_d[b, :, :])
                    nc.sync.dma_start(out=X2[64:128, 0:t_steps], in_=x_d[b, :, :])
                    nc.vector.memset(X2[0:64, 0:1], 0.0)
                    nc.vector.memset(X2[0:64, t_steps + 1 : t_steps + 2], 0.0)
                    nc.vector.memset(X2[64:128, t_steps : t_steps + 2], 0.0)
                    X2R = persist.tile([128, t_steps + 2], F32R, name=f"x2r{b}")
                    nc.vector.tensor_copy(X2R, X2)
                    X2Rs.append(X2R)

            # ---------- emission units ----------
            conv_state = {}

            def conv_mm(b, th, fp):
                ps = ppc.tile([128, THW], F32, tag="cps", name="cps")
                X2R = X2Rs[b]
                nc.tensor.matmul(
                    ps, convA[:, fp * 128 : (fp + 1) * 128],
                    X2R[:, th * THW : th * THW + THW],
                    start=True, stop=False,
                )
                nc.tensor.matmul(
                    ps, convB[:, fp * 128 : (fp + 1) * 128],
                    X2R[0:64, th * THW + 2 : th * THW + THW + 2],
                    start=False, stop=True,
                )
                if fp == 0:
                    macc = work.tile([128, THW], F32, tag="macc", name="macc")
                    conv_state[(b, th)] = macc
                    nc.vector.tensor_copy(macc, ps)
                else:
                    nc.vector.tensor_max(conv_state[(b, th)],
                                         conv_state[(b, th)], ps)

            def conv_tail(b, th):
                macc = conv_state.pop((b, th))
                mhi = work.tile([64, THW], F32, tag="mhi", name="mhi")
                nc.vector.tensor_copy(mhi, macc[64:128, :])
                m2 = work.tile([64, THW], F32, tag="m2", name="m2")
                nc.vector.tensor_max(m2, macc[0:64, :], mhi)
                out_ap = bigU[64:128, th * THW * nb + b : (th * THW + THW) * nb : nb]
                nc.scalar.activation(out_ap, m2, AF.Relu, bias=cb)

            def xn_unit(j):
                ps = ppm.tile([H, JW], F32, tag="mps", name="xnps")
                nc.tensor.matmul(
                    ps, w_in, bigU[64:128, j * JW : (j + 1) * JW],
                    start=True, stop=False,
                )
                nc.tensor.matmul(ps, b_in, ones, start=False, stop=True)
                nc.scalar.copy(bigH[64:128, j * JW : (j + 1) * JW], ps)

            def fc_unit(j):
                ps = ppm.tile([OUT, JW], F32, tag="mps", name="fcps")
                nc.tensor.matmul(
                    ps, fc_w, bigH[0:64, nb + j * JW : nb + (j + 1) * JW],
                    start=True, stop=False,
                )
                nc.tensor.matmul(ps, fc_b, ones, start=False, stop=True)
                ob = work.tile([OUT, JW], F32, tag="ob", name="ob")
                nc.scalar.copy(ob, ps)
                tpj = JW // nb
                for b in range(nb):
                    nc.sync.dma_start(
                        out=out_d[b, 0:OUT, j * tpj : (j + 1) * tpj],
                        in_=ob[:, b : JW : nb],
                    )

            def scan_step(k, prev_v):
                col = slice(k * nb, (k + 1) * nb)
                ncol = slice((k + 1) * nb, (k + 2) * nb)
                psum_rz = pps.tile([128, nb], F32, tag="rz", name="rz")
                psum_hn = pps.tile([H, nb], F32, tag="hn", name="hn")
                nc.tensor.matmul(psum_rz, w_rz, bigU[:, col], start=True, stop=False)
                nc.tensor.matmul(psum_hn, w_n, bigH[0:64, col], start=True, stop=True)
                nc.tensor.matmul(psum_rz, w_rz[0:64, :], prev_v, start=False, stop=True)

                r_s = scanw.tile([H, nb], F32, tag="rs", name="rs")
                nc.scalar.activation(r_s, psum_rz[0:64, :], AF.Sigmoid,
                                     bias=b_rz[0:64, :])
                z_s = scanw.tile([H, nb], F32, tag="zs", name="zs")
                nc.scalar.activation(z_s, psum_rz[64:128, :], AF.Sigmoid,
                                     bias=b_rz[64:128, :])
                zb_s = scanw.tile([H, nb], F32, tag="zbs", name="zbs")
                nc.scalar.activation(zb_s, psum_rz[64:128, :], AF.Sigmoid,
                                     bias=b_rz_neg, scale=-1.0)
                nc.vector.tensor_mul(bigU[0:64, ncol], z_s, bigH[0:64, col])
                q = scanw.tile([128, nb], F32, tag="q", name="q")
                nc.vector.scalar_tensor_tensor(
                    out=q[64:128, :], in0=psum_hn, scalar=b_hn, in1=r_s,
                    op0=OP.add, op1=OP.mult,
                )
                q2 = scanw.tile([H, nb], F32, tag="q2", name="q2")
                nc.vector.tensor_add(q2, q[64:128, :], bigH[64:128, col])
                n_t = scanw.tile([H, nb], F32, tag="n", name="n")
                nc.scalar.activation(n_t, q2, AF.Tanh)
                v_t = scanw.tile([H, nb], F32, tag="v", name="v")
                nc.vector.tensor_mul(v_t, zb_s, n_t)
                nc.vector.tensor_add(bigH[0:64, ncol], bigU[0:64, ncol], v_t)
                return v_t

            rep_ctx = tc.For_i(0, reps, 1) if reps > 1 else contextlib.nullcontext()
            with rep_ctx:
                if not inter:
                    for b in range(nb if "conv" in phases else 0):
                        for th in range(NTH):
                            for fp in range(NFP):
                                conv_mm(b, th, fp)
                            conv_tail(b, th)
                    for j in range(NJ if "xn" in phases else 0):
                        xn_unit(j)
                    prev_v = v_zero
                    for k in range(t_steps if "scan" in phases else 0):
                        prev_v = scan_step(k, prev_v)
                    for j in range(NJ if "fc" in phases else 0):
                        fc_unit(j)
                else:
                    # th=0 conv upfront + first-half xn
                    for b in range(nb):
                        for fp in range(NFP):
                            conv_mm(b, 0, fp)
                        conv_tail(b, 0)
                    for j in range(NJ // 2):
                        xn_unit(j)

                    # conv th=1 spread over scan steps [8, 440); 2nd-half xn
                    # after it; each fc tile as soon as its h-range is done.
                    units = []
                    for b in range(nb):
                        for fp in range(NFP):
                            units.append(("mm", b, fp))
                        units.append(("tail", b))
                    sched = {}
                    lo, hi = 8, 440
                    for i, u in enumerate(units):
                        k_at = lo + (i * (hi - lo)) // len(units)
                        sched.setdefault(k_at, []).append(u)
                    for j in range(NJ // 2, NJ):
                        sched.setdefault(444 + 8 * (j - NJ // 2), []).append(("xn", j))
                    tpj = JW // nb
                    for j in range(NJ):
                        k_at = (j + 1) * tpj
                        if k_at < t_steps:
                            sched.setdefault(k_at, []).append(("fc", j))

                    prev_v = v_zero
                    for k in range(t_steps):
                        prev_v = scan_step(k, prev_v)
                        for u in sched.get(k, ()):
                            if u[0] == "mm":
                                conv_mm(u[1], 1, u[2])
                            elif u[0] == "tail":
                                conv_tail(u[1], 1)
                            elif u[0] == "xn":
                                xn_unit(u[1])
                            elif u[0] == "fc":
                                fc_unit(u[1])
                    for j in range(NJ):
                        if (j + 1) * tpj >= t_steps:
                            fc_unit(j)

    nc.finalize()
    return nc


def prep_weights(conv_w, conv_b, w_ih, w_hh, b_ih, b_hh, fc_w, fc_b):
    """Host-side rearrangement of the small weights into device layouts."""
    conv_w = np.asarray(conv_w, np.float32)
    A = np.zeros((128, NFP * 128), np.float32)
    Bm = np.zeros((64, NFP * 128), np.float32)
    for fp in range(NFP):
        for fo in range(2):
            fout = 2 * fp + fo
            for fprime in range(max(0, fout - 1), min(64, fout + 2)):
                i = fprime - fout + 1
                cols = slice(fp * 128 + fo * 64, fp * 128 + fo * 64 + 64)
                A[fprime, cols] = conv_w[:, 0, i, 0]
                A[64 + fprime, cols] = conv_w[:, 0, i, 1]
                Bm[fprime, cols] = conv_w[:, 0, i, 2]
    w_ih = np.asarray(w_ih, np.float32)
    w_hh = np.asarray(w_hh, np.float32)
    b_ih = np.asarray(b_ih, np.float32)
    b_hh = np.asarray(b_hh, np.float32)
    return {
        "convA": A,
        "convB": Bm,
        "conv_bias": np.asarray(conv_b, np.float32).reshape(C, 1),
        "w_rz_lhsT": np.concatenate(
            [w_hh[0:128, :].T, w_ih[0:128, :].T], axis=0
        ).astype(np.float32).copy(),
        "w_n_lhsT": w_hh[128:192, :].T.astype(np.float32).copy(),
        "w_in_lhsT": w_ih[128:192, :].T.astype(np.float32).copy(),
        "b_rz": (b_ih[0:128] + b_hh[0:128]).reshape(128, 1).astype(np.float32),
        "b_rz_neg": (-(b_ih[64:128] + b_hh[64:128])).reshape(H, 1).astype(np.float32),
        "b_hn": b_hh[128:192].reshape(H, 1).astype(np.float32),
        "b_in_row": b_ih[128:192].reshape(1, H).astype(np.float32),
        "fc_lhsT": np.asarray(fc_w, np.float32).T.copy(),
        "fc_b_row": np.asarray(fc_b, np.float32).reshape(1, OUT),
    }


_NC_CACHE = {}


def _get_nc():
    if "nc" not in _NC_CACHE:
        _NC_CACHE["nc"] = build_crnn()
    return _NC_CACHE["nc"]


def run(inputs, trace=False):
    """Returns (out [B, OUT, T], BassKernelResults)."""
    x = np.asarray(inputs["x"], np.float32)
    wd = prep_weights(
        inputs["conv_w"], inputs["conv_b"], inputs["w_ih"], inputs["w_hh"],
        inputs["b_ih"], inputs["b_hh"], inputs["fc_w"], inputs["fc_b"],
    )
    nc = _get_nc()
    in_maps = []
    for i in range(NCORES):
        m = dict(wd)
        m["x"] = np.ascontiguousarray(x[i * NB : (i + 1) * NB])
        in_maps.append(m)
    res = run_bass_kernel_spmd(nc, in_maps, list(range(NCORES)), trace=trace)
    out = np.concatenate([res.results[i]["out"] for i in range(NCORES)], axis=0)
    return out, res


def kernel(**inputs) -> np.ndarray:
    out, _ = run(inputs, trace=False)
    return out

